# revision 1
# baseline (speedup 1.0000x reference)
"""Chamfer distance loss on Trainium2 (Bass/Tile), 8-core SPMD.

Reference math per batch b (inp/tgt: (B, C, N), mask: (B, N)):
    x = inp[b].T * mask[b,:,None]   # (N, 3)
    y = tgt[b].T * mask[b,:,None]
    d[n,m]  = ||x_n||^2 + ||y_m||^2 - 2 x_n.y_m
    loss    = mean(min_m d) + mean(min_n d)      (means over all B*N)

Device decomposition (data-parallel, 2 batches per core), using linearity
of the mean to split the norm terms out of the min:
    sum_n dist1 = sum_n min_m (y2[m] - 2 x_n.y_m) + sum_n x2[n]
    sum_m dist2 = sum_m min_n (x2[n] - 2 x_n.y_m) + sum_m y2[m]
Each min_m pass is a K=4 augmented matmul  [x0,x1,x2,1]^T . [-2y0,-2y1,-2y2,y2]
producing g[n,m] rows in PSUM, reduced with row-min only (no partition
reductions anywhere).  Row mins use the fused DVE tensor_tensor_reduce
(min elementwise of a PSUM half and an ACT-copied SBUF half, then min
along the free axis), so DVE and ACT split the reduction bandwidth.
Matmuls run as float32r (full PE rate at free dim 512, vs 4x slower fp32).

Host: shard batches across 8 cores, run SPMD, sum the per-core partial
sums and divide by B*N.
"""

import numpy as np

B, C, N = 16, 3, 4096
NCORES = 8
BPC = B // NCORES        # batches per core
NT = N // 128            # 32 n-tiles per pass
HALF = N // 2            # 2048 = one 4-bank PSUM group
BIG = float(np.finfo(np.float32).max)

_CACHE = {}


def _build():
    """Build the single-core Bass program (same program runs on all 8 cores
    with different input data)."""
    from contextlib import ExitStack

    from concourse import bacc, bass, mybir, tile  # noqa: F401

    f32 = mybir.dt.float32
    f32r = mybir.dt.float32r
    Alu = mybir.AluOpType

    nc = bacc.Bacc(trn_type="TRN2", target_bir_lowering=False, debug=False)

    inp_d = nc.dram_tensor("inp", [BPC, C, N], f32, kind="ExternalInput").ap()
    tgt_d = nc.dram_tensor("tgt", [BPC, C, N], f32, kind="ExternalInput").ap()
    mask_d = nc.dram_tensor("mask", [BPC, N], f32, kind="ExternalInput").ap()
    # Per-partition partial sums; host sums all of them and divides by B*N.
    out_d = nc.dram_tensor("out", [128, 1], f32, kind="ExternalOutput").ap()

    with tile.TileContext(nc) as tc, ExitStack() as ctx:
        pool = ctx.enter_context(tc.tile_pool(name="main", bufs=1))

        # Every compute op's partition pattern must start in row group 0
        # (offset < stride) or at 32/64/96, so operands live directly in
        # row-group-aligned mega-tiles, pass pb at partitions 32pb..32pb+3:
        #   wt group pb: rows 0-2 = -2*masked data, row 3 = 1.0   (stationary)
        #   rt group pb: rows 0-2 =    masked data, row 3 = norm  (moving)
        # Pass pairing (lhsT.T @ rhs = norm[m] - 2 v.w):
        #   pb0: -2x(b0) . y(b0)   pb1: -2y(b0) . x(b0)   pb2/pb3: batch 1
        # Final operands are float32r typed: the rounding copy below is their
        # sole writer, which satisfies the BIR verifier's "rounded to FP32r"
        # requirement for fp32r matmul inputs.
        wtr = pool.tile([128, N], f32r)
        rtr = pool.tile([128, N], f32r)
        # Columns 0..127: row-min of each (pass, n-tile)'s first PSUM half;
        # columns 128..255: second half; column 256: norm sums (rows 0-3).
        mincols = pool.tile([128, 8 * NT + 1], f32)
        fmin = pool.tile([128, 4 * NT], f32)
        r1a = pool.tile([128, 1], f32)
        r1 = pool.tile([128, 1], f32)

        # Min columns start at +BIG (neutral for the final min-combine — the
        # ACT/bf16-merged tiles only write their half-0 column); the norm-sum
        # column starts at 0 (neutral for the final add).
        nc.gpsimd.memset(mincols[:, 0 : 8 * NT], BIG)
        nc.gpsimd.memset(mincols[:, 8 * NT : 8 * NT + 1], 0.0)

        with tc.tile_pool(name="prep", bufs=1) as prpool:
            wt = prpool.tile([128, N], f32)
            rt = prpool.tile([128, N], f32)
            mr = prpool.tile([128, N], f32)  # per-batch mask rows (data rows only)
            dd = prpool.tile([4, 3 * N], f32)  # group pb's 3 masked rows, concat
            n4 = prpool.tile([4, N], f32)    # norm rows, group-major
            ones4 = prpool.tile([4, N], f32)

            nc.gpsimd.memset(ones4[:], 1.0)

            # Raw loads; group sources: wt <- (x0,y0,x1,y1), rt <- (y0,x0,y1,x1)
            # Only rows the matmuls read (32pb+0..3) are ever written/read, so
            # wt/rt need no zero-fill and every DMA below has at most 1 wait
            # (the DMA descriptor path supports only a single sync wait).
            for pb, b in enumerate((0, 0, 1, 1)):
                wsrc = inp_d[b] if pb % 2 == 0 else tgt_d[b]
                rsrc = tgt_d[b] if pb % 2 == 0 else inp_d[b]
                nc.gpsimd.dma_start(out=wt[32 * pb : 32 * pb + 3, :], in_=wsrc)
                nc.gpsimd.dma_start(out=rt[32 * pb : 32 * pb + 3, :], in_=rsrc)
                # DRAM-source broadcast AP: one DMA replicates the mask row.
                nc.gpsimd.dma_start(
                    out=mr[32 * pb : 32 * pb + 3, :],
                    in_=mask_d[b : b + 1, :].broadcast_to((3, N)),
                )

            # Mask data rows; -2 scale on wt's data rows; ones rows via DMA.
            for pb in range(4):
                g = slice(32 * pb, 32 * pb + 3)
                nc.vector.tensor_mul(rt[g, :], rt[g, :], mr[g, :])
                nc.vector.tensor_mul(wt[g, :], wt[g, :], mr[g, :])
                nc.scalar.mul(wt[g, :], wt[g, :], -2.0)
                nc.gpsimd.dma_start(
                    out=wt[32 * pb + 3 : 32 * pb + 4, :], in_=ones4[pb : pb + 1, :]
                )

            # Norm rows: gather each group's 3 masked rows into partition pb
            # (concatenated along free dim), square in place, sum the spans.
            for pb in range(4):
                nc.gpsimd.dma_start(
                    out=dd[pb : pb + 1, :], in_=rt[32 * pb : 32 * pb + 3, :]
                )
            nc.vector.tensor_mul(dd[:], dd[:], dd[:])
            nc.vector.tensor_add(n4[:], dd[:, 0:N], dd[:, N : 2 * N])
            nc.vector.tensor_add(n4[:], n4[:], dd[:, 2 * N : 3 * N])
            for pb in range(4):
                nc.gpsimd.dma_start(
                    out=rt[32 * pb + 3 : 32 * pb + 4, :], in_=n4[pb : pb + 1, :]
                )
            # sum_n x2 + sum_m y2 terms (linearity of the mean) go straight
            # into mincols' extra column.
            nc.vector.tensor_reduce(
                mincols[0:4, 8 * NT : 8 * NT + 1],
                n4[:],
                axis=mybir.AxisListType.X,
                op=Alu.add,
            )

            # Rounding copies into the fp32r operand tiles, per group (the
            # in-between rows are never read).
            for pb in range(4):
                g4 = slice(32 * pb, 32 * pb + 4)
                nc.vector.tensor_copy(wtr[g4, :], wt[g4, :])
                nc.scalar.copy(rtr[g4, :], rt[g4, :])

        ppool = ctx.enter_context(tc.tile_pool(name="psum", bufs=2, space="PSUM"))
        spool = ctx.enter_context(tc.tile_pool(name="scopy", bufs=4))
        mpool = ctx.enter_context(tc.tile_pool(name="merge", bufs=2))
        bf16 = mybir.dt.bfloat16

        # Reduction split: for 15/16 of tile-pairs, the otherwise-idle ACT
        # copy-converts both PSUM halves to bf16 SBUF and DVE merges them with
        # a 2x-mode bf16 tensor_tensor min before a single reduce (3.4us DVE
        # vs 4.5us direct); the rest reduce straight from PSUM.  Balances
        # DVE ~ ACT occupancy.
        for pb in range(4):
            bp = 32 * pb
            for t in range(NT):
                lhsT = wtr[bp : bp + 4, t * 128 : (t + 1) * 128]
                col = pb * NT + t
                halves = []
                for h in range(2):
                    ph = ppool.tile([128, HALF], f32, tag="ps", name="ph")
                    for j in range(4):
                        nc.tensor.matmul(
                            ph[:, j * 512 : (j + 1) * 512],
                            lhsT,
                            rtr[
                                bp : bp + 4,
                                h * HALF + j * 512 : h * HALF + (j + 1) * 512,
                            ],
                            start=True,
                            stop=True,
                            tile_position=(bp, 0),
                        )
                    halves.append(ph)
                if col % 16 == 0:
                    for h, ph in enumerate(halves):
                        nc.vector.tensor_reduce(
                            mincols[:, h * 128 + col : h * 128 + col + 1],
                            ph[:],
                            axis=mybir.AxisListType.X,
                            op=Alu.min,
                        )
                else:
                    s0 = spool.tile([128, HALF], bf16, tag="sc", name="s0")
                    s1 = spool.tile([128, HALF], bf16, tag="sc", name="s1")
                    nc.scalar.copy(s0[:], halves[0][:])
                    nc.scalar.copy(s1[:], halves[1][:])
                    mg = mpool.tile([128, HALF], bf16, tag="mg", name="mg")
                    nc.vector.tensor_tensor(mg[:], s0[:], s1[:], op=Alu.min)
                    nc.vector.tensor_reduce(
                        mincols[:, col : col + 1],
                        mg[:],
                        axis=mybir.AxisListType.X,
                        op=Alu.min,
                    )

        # Combine the two halves' mins, sum everything per partition, and add
        # the norm-sum column.
        nc.vector.tensor_tensor(
            fmin[:], mincols[:, 0:128], mincols[:, 128:256], op=Alu.min
        )
        nc.vector.tensor_reduce(
            r1a[:], fmin[:], axis=mybir.AxisListType.X, op=Alu.add
        )
        nc.vector.tensor_add(r1[:], r1a[:], mincols[:, 256:257])
        nc.gpsimd.dma_start(out=out_d[:], in_=r1[:])

    nc.compile()
    return nc


def _get_nc():
    if "nc" not in _CACHE:
        _CACHE["nc"] = _build()
    return _CACHE["nc"]


def _in_maps(inp, tgt, mask):
    inp = np.ascontiguousarray(inp, dtype=np.float32)
    tgt = np.ascontiguousarray(tgt, dtype=np.float32)
    mask = np.ascontiguousarray(mask, dtype=np.float32)
    return [
        {
            "inp": inp[c * BPC : (c + 1) * BPC],
            "tgt": tgt[c * BPC : (c + 1) * BPC],
            "mask": mask[c * BPC : (c + 1) * BPC],
        }
        for c in range(NCORES)
    ]


def _run(in_maps, **kwargs):
    from concourse.bass_utils import run_bass_kernel_spmd

    return run_bass_kernel_spmd(_get_nc(), in_maps, list(range(NCORES)), **kwargs)


def kernel(inp, tgt, mask):
    res = _run(_in_maps(inp, tgt, mask))
    total = 0.0
    for r in res.results:
        total += float(r["out"].sum())
    return np.float32(total / (B * N))



# revision 8
# speedup vs baseline: 1.3397x; 1.3397x over previous
"""Chamfer distance loss on Trainium2 (Bass/Tile), 8-core SPMD — v2.

Math per batch b (inp/tgt: (B, C, N), mask: (B, N)):
    x = inp[b].T * mask[b,:,None]   # (N, 3)
    y = tgt[b].T * mask[b,:,None]
    d[n,m] = ||x_n||^2 + ||y_m||^2 - 2 x_n.y_m
    loss   = mean(min_m d) + mean(min_n d)     (means over all B*N)

v2 design (vs the 584us baseline):
  * K=5 augmented f32r matmuls produce the full NEGATED distance matrix
    e = -d directly (norm terms folded into the matmul), in both layouts:
    pass A tiles are (n x m), pass B tiles are (m x n).  All reductions
    become MAX ops (min d = -max e), which every engine op supports.
  * Row maxes (the free axis) run on DVE as fused tensor_tensor_reduce
    (max elementwise of two adjacent 1024-column PSUM quarters, then max
    along the free axis) - one op covers 2048 columns.
  * Column maxes run on Pool (gpsimd) as partition_all_reduce(max) over
    the first MP columns of every pass-A tile; this REPLACES the first
    MP//128 pass-B tiles entirely, cutting PE and DVE work.  Per-tile
    results are DMA-hopped (idle SP engine + DMA) onto the partitions of
    a [32, MP] stack, and a second partition_all_reduce finishes the
    column direction per batch.
  * PSUM is one [128, 4096] region used as 4 rotating 1024-column
    quarters, so the PE never write-after-read stalls on a consumer.
  * ACT does all the operand prep (x2/y2 norm rows via Square, the
    2*masked moving-operand copies, norm/one row broadcasts), keeping
    DVE/Pool for the N^2 reductions.

Host: shard batches across 8 cores (2 each), run SPMD, sum the per-core
partial sums, negate, divide by B*N.
"""

import numpy as np

B, C, N = 16, 3, 4096
NCORES = 8
BPC = B // NCORES        # batches per core
NT = N // 128            # 32 tiles per pass
QW = 1024                # PSUM quarter width (f32)
MPT = 16                 # pass-B tiles replaced by Pool column-reduction
MP = 128 * MPT           # Pool-covered m-columns (from pass-A tiles)
BIG = float(np.finfo(np.float32).max)

_CACHE = {}


def _build():
    from contextlib import ExitStack

    from concourse import bacc, bass, bass_isa, mybir, tile  # noqa: F401

    f32 = mybir.dt.float32
    f32r = mybir.dt.float32r
    bf16 = mybir.dt.bfloat16
    Alu = mybir.AluOpType
    Act = mybir.ActivationFunctionType
    RO = bass_isa.ReduceOp

    nc = bacc.Bacc(trn_type="TRN2", target_bir_lowering=False, debug=False)

    inp_d = nc.dram_tensor("inp", [BPC, C, N], f32, kind="ExternalInput").ap()
    tgt_d = nc.dram_tensor("tgt", [BPC, C, N], f32, kind="ExternalInput").ap()
    mask_d = nc.dram_tensor("mask", [BPC, N], f32, kind="ExternalInput").ap()
    # row 0 = +1, row 1 = -1 (host-provided constants; f32r so the row
    # DMAs into the operand tiles are cast-free)
    ones_d = nc.dram_tensor("ones", [2, N], f32r, kind="ExternalInput").ap()
    # col 0: per-partition row-direction sums; [0,1]: Pool column sums.
    out_d = nc.dram_tensor("out", [128, 2], f32, kind="ExternalOutput").ap()

    # Units: (group, tile).  Groups at partition 32g: g0=(A,b0) lhsT=x(b0),
    # g1=(B,b0) lhsT=y(b0), g2=(A,b1), g3=(B,b1).  Pass-B tiles below MPT
    # are covered by the Pool column pass instead.  A (Pool-fed) and B
    # (DVE-only) units are interleaved evenly so both reducers stay busy.
    a_units = []
    b_units = []
    for k in range(NT // 2):
        a_units.append((0, 2 * k))
        a_units.append((0, 2 * k + 1))
        a_units.append((2, 2 * k))
        a_units.append((2, 2 * k + 1))
    for t in range(MPT, NT):
        b_units.append((1, t))
        b_units.append((3, t))
    units = []
    bi = 0
    for i, a in enumerate(a_units):
        units.append(a)
        want = ((i + 1) * len(b_units)) // len(a_units)
        while bi < want:
            units.append(b_units[bi])
            bi += 1
    units.extend(b_units[bi:])
    U = len(units)

    with tile.TileContext(nc) as tc, ExitStack() as ctx:
        pool = ctx.enter_context(tc.tile_pool(name="main", bufs=1))

        wtr = pool.tile([128, N], f32r)   # stationary: coords/norm/one rows
        rtr = pool.tile([128, N], f32r)   # moving: 2*coords/-1/-norm rows
        psum = ctx.enter_context(
            tc.tile_pool(name="ps", bufs=1, space="PSUM")
        ).tile([128, N], f32)
        dcols = pool.tile([128, 2 * U], f32)   # ttr accums (2 per unit)
        stack = [pool.tile([NT, MP], bf16, name=f"stack{b}") for b in range(BPC)]
        pout = pool.tile([1, BPC + 1], f32)
        dtot = pool.tile([128, 1], f32)
        dmax = pool.tile([128, U], f32)

        with tc.tile_pool(name="prep", bufs=1) as prpool:
            wraw = prpool.tile([128, N], f32)  # raw coords (rows 32g..+3)
            mr = prpool.tile([128, N], f32)    # broadcast mask rows
            n4g = [prpool.tile([1, N], f32r, name=f"n4g{g}") for g in range(4)]
            sqz = [prpool.tile([3, N], f32, name=f"sqz{g}") for g in range(4)]

            # All input DMAs dispatch first (split over the idle SP and
            # Pool sequencers) so no load queues behind a compute-waiting
            # op.  Then per group: mask-mul (DVE, the only prep DVE work),
            # squares of the 3 coord rows (ACT, into the dead wraw rows),
            # 3-channel partition-add (Pool, into the dead mr rows), a
            # rounding copy (ACT, ordered after all squares/muls so its
            # Pool wait never head-of-line blocks them), then DMAs place
            # the aug rows.
            dsp = [nc.sync, nc.gpsimd]
            for g in range(4):
                p, i = g // 2, g % 2
                src = inp_d[p] if i == 0 else tgt_d[p]
                dsp[i].dma_start(out=wraw[32 * g : 32 * g + 3, :], in_=src)
                dsp[i].dma_start(
                    out=mr[32 * g : 32 * g + 3, :],
                    in_=mask_d[p : p + 1, :].broadcast_to((3, N)),
                )
            for pp in range(2):
                for g in (2 * pp, 2 * pp + 1):
                    r = slice(32 * g, 32 * g + 3)
                    nc.vector.tensor_mul(wtr[r, :], wraw[r, :], mr[r, :])
                    nc.scalar.activation(wraw[r, :], wtr[r, :], Act.Square)
                    # partition_all_reduce only works from partition 0 on
                    # real HW: hop the squared rows down first
                    dsp[g % 2].dma_start(out=sqz[g][:], in_=wraw[r, :])
                    nc.gpsimd.partition_all_reduce(
                        sqz[g][:], sqz[g][:], 3, RO.add
                    )
                # Moving-side data rows on the (startup-idle) DVE:
                # rtr rows0-2 = 2 * other-group coords.
                for g in (2 * pp, 2 * pp + 1):
                    o = g ^ 1
                    nc.vector.tensor_scalar_mul(
                        rtr[32 * g : 32 * g + 3, :],
                        wtr[32 * o : 32 * o + 3, :],
                        2.0,
                    )
            # Aug norm/const rows (e = 2x.y - x2 - y2, all via + rows and
            # two -1 constants): wtr row3 = own norm, row4 = -1; rtr
            # row3 = -1, row4 = other norm.
            for g in range(4):
                nc.scalar.copy(n4g[g][:], sqz[g][0:1, :])
            for g in range(4):
                i = g % 2
                o = g ^ 1
                dsp[i].dma_start(
                    out=wtr[32 * g + 3 : 32 * g + 4, :], in_=n4g[g][:]
                )
                dsp[i].dma_start(
                    out=rtr[32 * g + 4 : 32 * g + 5, :], in_=n4g[o][:]
                )
                dsp[i].dma_start(
                    out=wtr[32 * g + 4 : 32 * g + 5, :], in_=ones_d[1:2, :]
                )
                dsp[i].dma_start(
                    out=rtr[32 * g + 3 : 32 * g + 4, :], in_=ones_d[1:2, :]
                )

        spool = ctx.enter_context(tc.tile_pool(name="scr", bufs=4))
        parpool = ctx.enter_context(tc.tile_pool(name="par", bufs=3))
        parcpool = ctx.enter_context(tc.tile_pool(name="parc", bufs=4))
        parc2pool = ctx.enter_context(tc.tile_pool(name="parc2", bufs=3))
        scbpool = ctx.enter_context(tc.tile_pool(name="scb", bufs=3))

        # Quarter stream: each (g, t) unit is four 1024-column quarters;
        # quarter k always lands in PSUM slot k (depth-4 pipeline).  Every
        # PSUM range has exactly ONE reader so each matmul carries at most
        # one semaphore wait (multi-wait joins head-of-line block the PE
        # sequencer):
        #   A units: q0/q1 are read only by ACT copies into bf16 SBUF;
        #     DVE's first row-max ttr and Pool's column maxes both consume
        #     the copy.  q2/q3 are read only by the second row-max ttr.
        #   B units: both ttrs read PSUM directly.
        # Pool results for a PAIR of same-batch A-units collect (as bf16)
        # in one par tile, then one DMA hop moves both rows onto the
        # batch stack.
        par = None
        a_idx = 0
        for u, (g, t) in enumerate(units):
            gp = slice(32 * g, 32 * g + 5)
            lhsT = wtr[gp, t * 128 : (t + 1) * 128]
            isa = g % 2 == 0
            if isa and a_idx % 2 == 0:
                par = parpool.tile([128, 2 * MP], bf16, tag="par", name="par")
            off = (a_idx % 2) * MP
            if isa:
                parc = parcpool.tile([128, MP], bf16, tag="parc", name="parc")
                parc2 = parc2pool.tile([128, QW], bf16, tag="parc2", name="parc2")
            # A units emit the PSUM-direct half (q2/q3) BEFORE the q0/q1
            # copy chain, so the DVE always has ready work in front of the
            # copy-dependent ops.  Row maxes use tensor_tensor_scan
            # (max,max) — the fused 2-input reduce; its last column is the
            # row max (tensor_tensor_reduce does not run on this HW).  At
            # most one scan input may be PSUM, so q2 is ACT-copied.
            qorder = (2, 3, 0, 1) if isa else (0, 1, 2, 3)
            for q in qorder:
                sb = q * QW              # PSUM slot base == m-column base
                for j in range(2):
                    nc.tensor.matmul(
                        psum[:, sb + j * 512 : sb + (j + 1) * 512],
                        lhsT,
                        rtr[gp, sb + j * 512 : sb + (j + 1) * 512],
                        start=True,
                        stop=True,
                        tile_position=(32 * g, 0),
                    )
                if isa and q == 2:
                    # sole PSUM reader of q2: copy to bf16 SBUF
                    nc.scalar.copy(parc2[:], psum[:, sb : sb + QW])
                elif isa and q == 3:
                    # row max of the q2/q3 half: scan(q3 PSUM, q2 copy)
                    sc = spool.tile([128, QW], bf16, tag="sc", name="sc")
                    nc.vector.tensor_tensor_scan(
                        out=sc[:],
                        data0=psum[:, sb : sb + QW],
                        data1=parc2[:],
                        initial=-BIG,
                        op0=Alu.max,
                        op1=Alu.max,
                    )
                    nc.vector.tensor_copy(
                        dcols[:, 2 * u + 1 : 2 * u + 2], sc[:, QW - 1 : QW]
                    )
                elif isa and q < 2:
                    # sole PSUM reader of q0/q1: copy to bf16 SBUF
                    nc.scalar.copy(parc[:, sb : sb + QW], psum[:, sb : sb + QW])
                    if q == 1:
                        # row max of the copied half: all-SBUF scan
                        sc = spool.tile([128, QW], bf16, tag="sc", name="sc")
                        nc.vector.tensor_tensor_scan(
                            out=sc[:],
                            data0=parc[:, 0:QW],
                            data1=parc[:, QW : 2 * QW],
                            initial=-BIG,
                            op0=Alu.max,
                            op1=Alu.max,
                        )
                        nc.vector.tensor_copy(
                            dcols[:, 2 * u : 2 * u + 1], sc[:, QW - 1 : QW]
                        )
                        # column maxes of the copied half
                        for j in range(2):
                            nc.gpsimd.partition_all_reduce(
                                par[:, off + j * QW : off + (j + 1) * QW],
                                parc[:, j * QW : (j + 1) * QW],
                                128,
                                RO.max,
                            )
                elif q % 2 == 1:
                    # B units: plain full-width DVE reduce straight from
                    # PSUM (sole reader of both its slots).
                    h = q // 2
                    nc.vector.tensor_reduce(
                        dcols[:, 2 * u + h : 2 * u + h + 1],
                        psum[:, sb - QW : sb + QW],
                        axis=mybir.AxisListType.X,
                        op=Alu.max,
                    )
            if isa:
                if a_idx % 2 == 1:
                    # pair (t-1, t) complete: hop both rows at once
                    nc.sync.dma_start(
                        out=stack[g // 2][t - 1 : t + 1, :],
                        in_=par[0:1, 0 : 2 * MP],
                    )
                a_idx += 1

        # Column-direction finish: per batch, partition max of the 32
        # stacked rows, then sum over the MP columns.
        finpool = ctx.enter_context(tc.tile_pool(name="finp", bufs=1))
        for b in range(BPC):
            fin = finpool.tile([128, MP], bf16, tag="fin", name="fin")
            nc.gpsimd.partition_all_reduce(fin[0:NT, :], stack[b][:], NT, RO.max)
            nc.vector.tensor_reduce(
                pout[0:1, b : b + 1],
                fin[0:1, :],
                axis=mybir.AxisListType.X,
                op=Alu.add,
            )

        # Row-direction finish: per-unit max of its two half-row maxes,
        # then sum across units per partition.
        nc.vector.tensor_reduce(
            dmax[:],
            dcols[:].rearrange("p (u two) -> p u two", two=2),
            axis=mybir.AxisListType.X,
            op=Alu.max,
        )
        nc.vector.tensor_reduce(
            dtot[:], dmax[:], axis=mybir.AxisListType.X, op=Alu.add
        )
        nc.vector.tensor_reduce(
            pout[0:1, BPC : BPC + 1],
            pout[0:1, 0:BPC],
            axis=mybir.AxisListType.X,
            op=Alu.add,
        )
        nc.sync.dma_start(out=out_d[:, 0:1], in_=dtot[:])
        nc.sync.dma_start(out=out_d[0:1, 1:2], in_=pout[0:1, BPC : BPC + 1])

    nc.compile()
    return nc


def _get_nc():
    if "nc" not in _CACHE:
        _CACHE["nc"] = _build()
    return _CACHE["nc"]


def _in_maps(inp, tgt, mask):
    inp = np.ascontiguousarray(inp, dtype=np.float32)
    tgt = np.ascontiguousarray(tgt, dtype=np.float32)
    mask = np.ascontiguousarray(mask, dtype=np.float32)
    ones = np.empty((2, N), dtype=np.float32)
    ones[0] = 1.0
    ones[1] = -1.0
    return [
        {
            "inp": inp[c * BPC : (c + 1) * BPC],
            "tgt": tgt[c * BPC : (c + 1) * BPC],
            "mask": mask[c * BPC : (c + 1) * BPC],
            "ones": ones,
        }
        for c in range(NCORES)
    ]


def _run(in_maps, **kwargs):
    from concourse.bass_utils import run_bass_kernel_spmd

    return run_bass_kernel_spmd(_get_nc(), in_maps, list(range(NCORES)), **kwargs)


def kernel(inp, tgt, mask):
    res = _run(_in_maps(inp, tgt, mask))
    total = 0.0
    for r in res.results:
        o = r["out"]
        total += float(o[:, 0].sum()) + float(o[0, 1])
    return np.float32(-total / (B * N))


# revision 10
# speedup vs baseline: 1.7756x; 1.3254x over previous
"""Chamfer distance loss on Trainium2 (Bass/Tile), 8-core SPMD — v2.

Math per batch b (inp/tgt: (B, C, N), mask: (B, N)):
    x = inp[b].T * mask[b,:,None]   # (N, 3)
    y = tgt[b].T * mask[b,:,None]
    d[n,m] = ||x_n||^2 + ||y_m||^2 - 2 x_n.y_m
    loss   = mean(min_m d) + mean(min_n d)     (means over all B*N)

v2 design (vs the 584us baseline):
  * K=5 augmented f32r matmuls produce the full NEGATED distance matrix
    e = -d directly (norm terms folded into the matmul), in both layouts:
    pass A tiles are (n x m), pass B tiles are (m x n).  All reductions
    become MAX ops (min d = -max e), which every engine op supports.
  * Row maxes (the free axis) run on DVE as fused tensor_tensor_reduce
    (max elementwise of two adjacent 1024-column PSUM quarters, then max
    along the free axis) - one op covers 2048 columns.
  * Column maxes run on Pool (gpsimd) as partition_all_reduce(max) over
    the first MP columns of every pass-A tile; this REPLACES the first
    MP//128 pass-B tiles entirely, cutting PE and DVE work.  Per-tile
    results are DMA-hopped (idle SP engine + DMA) onto the partitions of
    a [32, MP] stack, and a second partition_all_reduce finishes the
    column direction per batch.
  * PSUM is one [128, 4096] region used as 4 rotating 1024-column
    quarters, so the PE never write-after-read stalls on a consumer.
  * ACT does all the operand prep (x2/y2 norm rows via Square, the
    2*masked moving-operand copies, norm/one row broadcasts), keeping
    DVE/Pool for the N^2 reductions.

Host: shard batches across 8 cores (2 each), run SPMD, sum the per-core
partial sums, negate, divide by B*N.
"""

import numpy as np

B, C, N = 16, 3, 4096
NCORES = 8
BPC = B // NCORES        # batches per core
NT = N // 128            # 32 tiles per pass
QW = 1024                # PSUM quarter width (f32)
MPT = 16                 # pass-B tiles replaced by Pool column-reduction
MP = 128 * MPT           # Pool-covered m-columns (from pass-A tiles)
BIG = float(np.finfo(np.float32).max)

_CACHE = {}


def _build():
    from contextlib import ExitStack

    from concourse import bacc, bass, bass_isa, mybir, tile  # noqa: F401

    f32 = mybir.dt.float32
    f32r = mybir.dt.float32r
    bf16 = mybir.dt.bfloat16
    Alu = mybir.AluOpType
    Act = mybir.ActivationFunctionType
    RO = bass_isa.ReduceOp

    nc = bacc.Bacc(trn_type="TRN2", target_bir_lowering=False, debug=False)

    inp_d = nc.dram_tensor("inp", [BPC, C, N], f32, kind="ExternalInput").ap()
    tgt_d = nc.dram_tensor("tgt", [BPC, C, N], f32, kind="ExternalInput").ap()
    mask_d = nc.dram_tensor("mask", [BPC, N], f32, kind="ExternalInput").ap()
    # row 0 = +1, row 1 = -1 (host-provided constants; f32r so the row
    # DMAs into the operand tiles are cast-free)
    ones_d = nc.dram_tensor("ones", [2, N], f32r, kind="ExternalInput").ap()
    # col 0: per-partition row-direction sums; [0,1]: Pool column sums.
    out_d = nc.dram_tensor("out", [128, 2], f32, kind="ExternalOutput").ap()

    # Units: (group, tile).  Groups at partition 32g: g0=(A,b0) lhsT=x(b0),
    # g1=(B,b0) lhsT=y(b0), g2=(A,b1), g3=(B,b1).  Pass-B tiles below MPT
    # are covered by the Pool column pass instead.  A (Pool-fed) and B
    # (DVE-only) units are interleaved evenly so both reducers stay busy.
    a_units = []
    b_units = []
    for k in range(NT // 2):
        a_units.append((0, 2 * k))
        a_units.append((0, 2 * k + 1))
        a_units.append((2, 2 * k))
        a_units.append((2, 2 * k + 1))
    for t in range(MPT, NT):
        b_units.append((1, t))
        b_units.append((3, t))
    units = []
    bi = 0
    for i, a in enumerate(a_units):
        units.append(a)
        want = ((i + 1) * len(b_units)) // len(a_units)
        while bi < want:
            units.append(b_units[bi])
            bi += 1
    units.extend(b_units[bi:])
    U = len(units)

    with tile.TileContext(nc) as tc, ExitStack() as ctx:
        pool = ctx.enter_context(tc.tile_pool(name="main", bufs=1))

        wtr = pool.tile([128, N], f32r)   # stationary: coords/norm/one rows
        rtr = pool.tile([128, N], f32r)   # moving: 2*coords/-1/-norm rows
        psum = ctx.enter_context(
            tc.tile_pool(name="ps", bufs=1, space="PSUM")
        ).tile([128, N], f32)
        dcols = pool.tile([128, 2 * U], f32)   # ttr accums (2 per unit)
        stack = [pool.tile([NT, MP], bf16, name=f"stack{b}") for b in range(BPC)]
        pout = pool.tile([1, BPC + 1], f32)
        dtot = pool.tile([128, 1], f32)
        dmax = pool.tile([128, U], f32)

        with tc.tile_pool(name="prep", bufs=1) as prpool:
            wraw = prpool.tile([128, N], f32)  # raw coords (rows 32g..+3)
            mr = prpool.tile([128, N], f32)    # broadcast mask rows
            n4g = [prpool.tile([1, N], f32r, name=f"n4g{g}") for g in range(4)]
            sqz = [prpool.tile([3, N], f32, name=f"sqz{g}") for g in range(4)]

            # All input DMAs dispatch first (split over the idle SP and
            # Pool sequencers) so no load queues behind a compute-waiting
            # op.  Then per group: mask-mul (DVE, the only prep DVE work),
            # squares of the 3 coord rows (ACT, into the dead wraw rows),
            # 3-channel partition-add (Pool, into the dead mr rows), a
            # rounding copy (ACT, ordered after all squares/muls so its
            # Pool wait never head-of-line blocks them), then DMAs place
            # the aug rows.
            dsp = [nc.sync, nc.gpsimd]
            for g in range(4):
                p, i = g // 2, g % 2
                src = inp_d[p] if i == 0 else tgt_d[p]
                dsp[i].dma_start(out=wraw[32 * g : 32 * g + 3, :], in_=src)
                dsp[i].dma_start(
                    out=mr[32 * g : 32 * g + 3, :],
                    in_=mask_d[p : p + 1, :].broadcast_to((3, N)),
                )
            for pp in range(2):
                for g in (2 * pp, 2 * pp + 1):
                    r = slice(32 * g, 32 * g + 3)
                    nc.vector.tensor_mul(wtr[r, :], wraw[r, :], mr[r, :])
                    nc.scalar.activation(wraw[r, :], wtr[r, :], Act.Square)
                    # partition_all_reduce only works from partition 0 on
                    # real HW: hop the squared rows down first
                    dsp[g % 2].dma_start(out=sqz[g][:], in_=wraw[r, :])
                    nc.gpsimd.partition_all_reduce(
                        sqz[g][:], sqz[g][:], 3, RO.add
                    )
                # Moving-side data rows on the (startup-idle) DVE:
                # rtr rows0-2 = 2 * other-group coords.
                for g in (2 * pp, 2 * pp + 1):
                    o = g ^ 1
                    nc.vector.tensor_scalar_mul(
                        rtr[32 * g : 32 * g + 3, :],
                        wtr[32 * o : 32 * o + 3, :],
                        2.0,
                    )
            # Aug norm/const rows (e = 2x.y - x2 - y2, all via + rows and
            # two -1 constants): wtr row3 = own norm, row4 = -1; rtr
            # row3 = -1, row4 = other norm.
            for g in range(4):
                nc.scalar.copy(n4g[g][:], sqz[g][0:1, :])
            for g in range(4):
                i = g % 2
                o = g ^ 1
                dsp[i].dma_start(
                    out=wtr[32 * g + 3 : 32 * g + 4, :], in_=n4g[g][:]
                )
                dsp[i].dma_start(
                    out=rtr[32 * g + 4 : 32 * g + 5, :], in_=n4g[o][:]
                )
                dsp[i].dma_start(
                    out=wtr[32 * g + 4 : 32 * g + 5, :], in_=ones_d[1:2, :]
                )
                dsp[i].dma_start(
                    out=rtr[32 * g + 3 : 32 * g + 4, :], in_=ones_d[1:2, :]
                )

        spool = ctx.enter_context(tc.tile_pool(name="scr", bufs=4))
        parpool = ctx.enter_context(tc.tile_pool(name="par", bufs=3))
        parcpool = ctx.enter_context(tc.tile_pool(name="parc", bufs=4))
        parc2pool = ctx.enter_context(tc.tile_pool(name="parc2", bufs=4))
        scbpool = ctx.enter_context(tc.tile_pool(name="scb", bufs=3))

        # Quarter stream: each (g, t) unit is four 1024-column quarters;
        # quarter k always lands in PSUM slot k (depth-4 pipeline).  Every
        # PSUM range has exactly ONE reader so each matmul carries at most
        # one semaphore wait (multi-wait joins head-of-line block the PE
        # sequencer):
        #   A units: q0/q1 are read only by ACT copies into bf16 SBUF;
        #     DVE's first row-max ttr and Pool's column maxes both consume
        #     the copy.  q2/q3 are read only by the second row-max ttr.
        #   B units: both ttrs read PSUM directly.
        # Pool results for a PAIR of same-batch A-units collect (as bf16)
        # in one par tile, then one DMA hop moves both rows onto the
        # batch stack.
        par = None
        a_idx = 0
        for u, (g, t) in enumerate(units):
            gp = slice(32 * g, 32 * g + 5)
            lhsT = wtr[gp, t * 128 : (t + 1) * 128]
            isa = g % 2 == 0
            if isa and a_idx % 2 == 0:
                par = parpool.tile([128, 2 * MP], bf16, tag="par", name="par")
            off = (a_idx % 2) * MP
            if isa:
                parc = parcpool.tile([128, MP], bf16, tag="parc", name="parc")
                parc2 = parc2pool.tile([128, QW], bf16, tag="parc2", name="parc2")
            # A units emit the PSUM-direct half (q2/q3) BEFORE the q0/q1
            # copy chain, so the DVE always has ready work in front of the
            # copy-dependent ops.  Row maxes use tensor_tensor_scan
            # (max,max) — the fused 2-input reduce; its last column is the
            # row max (tensor_tensor_reduce does not run on this HW).  At
            # most one scan input may be PSUM, so q2 is ACT-copied.
            qorder = (2, 3, 0, 1) if isa else (0, 1, 2, 3)
            for q in qorder:
                sb = q * QW              # PSUM slot base == m-column base
                for j in range(2):
                    nc.tensor.matmul(
                        psum[:, sb + j * 512 : sb + (j + 1) * 512],
                        lhsT,
                        rtr[gp, sb + j * 512 : sb + (j + 1) * 512],
                        start=True,
                        stop=True,
                        tile_position=(32 * g, 0),
                    )
                if isa and q == 2:
                    # sole PSUM reader of q2: copy to bf16 SBUF
                    nc.scalar.copy(parc2[:], psum[:, sb : sb + QW])
                elif isa and q == 3:
                    # row max of the q2/q3 half: scan(q3 PSUM, q2 copy)
                    sc = spool.tile([128, QW], bf16, tag="sc", name="sc")
                    nc.vector.tensor_tensor_scan(
                        out=sc[:],
                        data0=psum[:, sb : sb + QW],
                        data1=parc2[:],
                        initial=-BIG,
                        op0=Alu.max,
                        op1=Alu.max,
                    )
                    nc.vector.tensor_copy(
                        dcols[:, 2 * u + 1 : 2 * u + 2], sc[:, QW - 1 : QW]
                    )
                elif isa and q < 2:
                    # sole PSUM reader of q0/q1: copy to bf16 SBUF
                    nc.scalar.copy(parc[:, sb : sb + QW], psum[:, sb : sb + QW])
                    if q == 1:
                        # row max of the copied half: all-SBUF scan
                        sc = spool.tile([128, QW], bf16, tag="sc", name="sc")
                        nc.vector.tensor_tensor_scan(
                            out=sc[:],
                            data0=parc[:, 0:QW],
                            data1=parc[:, QW : 2 * QW],
                            initial=-BIG,
                            op0=Alu.max,
                            op1=Alu.max,
                        )
                        nc.vector.tensor_copy(
                            dcols[:, 2 * u : 2 * u + 1], sc[:, QW - 1 : QW]
                        )
                        # column maxes of the copied half
                        for j in range(2):
                            nc.gpsimd.partition_all_reduce(
                                par[:, off + j * QW : off + (j + 1) * QW],
                                parc[:, j * QW : (j + 1) * QW],
                                128,
                                RO.max,
                            )
                elif q % 2 == 0:
                    # B units, even quarter: sole PSUM reader is an ACT
                    # copy (the idle ACT has ample slack)
                    parc2 = parc2pool.tile(
                        [128, QW], bf16, tag="parc2", name="parc2"
                    )
                    nc.scalar.copy(parc2[:], psum[:, sb : sb + QW])
                else:
                    # B units, odd quarter: fused scan(PSUM, copy)
                    h = q // 2
                    sc = spool.tile([128, QW], bf16, tag="sc", name="sc")
                    nc.vector.tensor_tensor_scan(
                        out=sc[:],
                        data0=psum[:, sb : sb + QW],
                        data1=parc2[:],
                        initial=-BIG,
                        op0=Alu.max,
                        op1=Alu.max,
                    )
                    nc.vector.tensor_copy(
                        dcols[:, 2 * u + h : 2 * u + h + 1], sc[:, QW - 1 : QW]
                    )
            if isa:
                if a_idx % 2 == 1:
                    # pair (t-1, t) complete: hop both rows at once
                    nc.sync.dma_start(
                        out=stack[g // 2][t - 1 : t + 1, :],
                        in_=par[0:1, 0 : 2 * MP],
                    )
                a_idx += 1

        # Column-direction finish: per batch, partition max of the 32
        # stacked rows, then sum over the MP columns.
        finpool = ctx.enter_context(tc.tile_pool(name="finp", bufs=1))
        for b in range(BPC):
            fin = finpool.tile([128, MP], bf16, tag="fin", name="fin")
            nc.gpsimd.partition_all_reduce(fin[0:NT, :], stack[b][:], NT, RO.max)
            nc.vector.tensor_reduce(
                pout[0:1, b : b + 1],
                fin[0:1, :],
                axis=mybir.AxisListType.X,
                op=Alu.add,
            )

        # Row-direction finish: per-unit max of its two half-row maxes,
        # then sum across units per partition.
        nc.vector.tensor_reduce(
            dmax[:],
            dcols[:].rearrange("p (u two) -> p u two", two=2),
            axis=mybir.AxisListType.X,
            op=Alu.max,
        )
        nc.vector.tensor_reduce(
            dtot[:], dmax[:], axis=mybir.AxisListType.X, op=Alu.add
        )
        nc.vector.tensor_reduce(
            pout[0:1, BPC : BPC + 1],
            pout[0:1, 0:BPC],
            axis=mybir.AxisListType.X,
            op=Alu.add,
        )
        nc.sync.dma_start(out=out_d[:, 0:1], in_=dtot[:])
        nc.sync.dma_start(out=out_d[0:1, 1:2], in_=pout[0:1, BPC : BPC + 1])

    nc.compile()
    return nc


def _get_nc():
    if "nc" not in _CACHE:
        _CACHE["nc"] = _build()
    return _CACHE["nc"]


def _in_maps(inp, tgt, mask):
    inp = np.ascontiguousarray(inp, dtype=np.float32)
    tgt = np.ascontiguousarray(tgt, dtype=np.float32)
    mask = np.ascontiguousarray(mask, dtype=np.float32)
    ones = np.empty((2, N), dtype=np.float32)
    ones[0] = 1.0
    ones[1] = -1.0
    return [
        {
            "inp": inp[c * BPC : (c + 1) * BPC],
            "tgt": tgt[c * BPC : (c + 1) * BPC],
            "mask": mask[c * BPC : (c + 1) * BPC],
            "ones": ones,
        }
        for c in range(NCORES)
    ]


def _run(in_maps, **kwargs):
    from concourse.bass_utils import run_bass_kernel_spmd

    return run_bass_kernel_spmd(_get_nc(), in_maps, list(range(NCORES)), **kwargs)


def kernel(inp, tgt, mask):
    res = _run(_in_maps(inp, tgt, mask))
    total = 0.0
    for r in res.results:
        o = r["out"]
        total += float(o[:, 0].sum()) + float(o[0, 1])
    return np.float32(-total / (B * N))
